# revision 1
# baseline (speedup 1.0000x reference)
"""nn_Attention Trainium2 Bass kernel.

Full attention forward: x->(q,k,v) with l2-normalized weights, per-head-dim
l2 norm + learned qk scale, interleaved RoPE, causal SDPA, output projection
with column-l2-normalized wo.

Sharding: TP=4 over heads (8 heads/core) x DP=2 over batch across 8 cores.
Each core computes a partial [2048, 2048] output for its batch; host sums
the 4 TP partials per batch.

Device layout tricks:
- rope-pair permutation (evens|odds blocks) folded into wq/wk rows host-side,
  so RoPE's rotate-half becomes a contiguous 32-column swap.
- qk_scale folded into the cos/sin tables host-side.
- transposed softmax: logitsT [sj, si] blocks; logits are bounded
  (|logit| <= max(qk_scale)^2), so exp without max subtraction is safe;
  causal mask applied as a 0/1 multiply on diagonal blocks; PV consumes
  attnT directly (no attention transposes); softmax denominators from a
  ones-column matmul alongside PV; 1/denom broadcast via K=1 matmul.
- attention output overwrites qT storage (disjoint column ranges already
  consumed), saving 4MB of SBUF.
"""
import sys
import os
import math
from contextlib import ExitStack

sys.path.insert(0, "/opt/trn_rl_repo")

import numpy as np
import ml_dtypes

BF16 = ml_dtypes.bfloat16

B, S, DIM = 2, 2048, 2048
HEADS, DH = 32, 64
THETA = 10000.0
NCORES = 8
TP = 4             # head-parallel ways
HPC = HEADS // TP  # heads per core = 8
E = HPC * DH       # per-core qkv width = 512
ET = E // 128      # e-tiles per core = 4
DT = DIM // 128    # contraction d-tiles = 16
SB = S // 512      # 512-wide seq blocks = 4
SS = S // 128      # 128-wide seq blocks = 16

_CACHE = {}


def _l2n(w, axis):
    n = np.sqrt((w.astype(np.float64) ** 2).sum(axis=axis, keepdims=True))
    n = np.maximum(n, 1e-12)
    return (w / n).astype(np.float32)


def _build_program():
    import concourse.bass as bass
    from concourse import bacc
    import concourse.mybir as mybir
    import concourse.tile as tile
    from concourse.masks import make_identity

    f32 = mybir.dt.float32
    bf16 = mybir.dt.bfloat16
    AF = mybir.ActivationFunctionType

    nc = bacc.Bacc("TRN2", target_bir_lowering=False)

    xT = nc.dram_tensor("xT", [DIM, S], bf16, kind="ExternalInput")
    wqT = nc.dram_tensor("wqT", [DIM, E], bf16, kind="ExternalInput")
    wkT = nc.dram_tensor("wkT", [DIM, E], bf16, kind="ExternalInput")
    wvT = nc.dram_tensor("wvT", [DIM, E], bf16, kind="ExternalInput")
    woT = nc.dram_tensor("woT", [E, DIM], bf16, kind="ExternalInput")
    cosd = nc.dram_tensor("cosd", [128, SS * DH], bf16, kind="ExternalInput")
    sind = nc.dram_tensor("sind", [128, SS * DH], bf16, kind="ExternalInput")
    maskd = nc.dram_tensor("maskd", [128, 4 * 512], bf16, kind="ExternalInput")
    Y = nc.dram_tensor("Y", [S, DIM], f32, kind="ExternalOutput")

    with tile.TileContext(nc) as tc, ExitStack() as ctx:
        const = ctx.enter_context(tc.tile_pool(name="const", bufs=1))
        wpool = ctx.enter_context(tc.tile_pool(name="wpool", bufs=4))
        xpool = ctx.enter_context(tc.tile_pool(name="xpool", bufs=1))
        qkv = ctx.enter_context(tc.tile_pool(name="qkv", bufs=1))
        work = ctx.enter_context(tc.tile_pool(name="work", bufs=1))
        attnp = ctx.enter_context(tc.tile_pool(name="attnp", bufs=3))
        ypool = ctx.enter_context(tc.tile_pool(name="ypool", bufs=1))

        # constants
        cos_sb = const.tile([128, SS, DH], bf16)
        sin_sb = const.tile([128, SS, DH], bf16)
        mask_sb = const.tile([128, 4, 512], bf16)
        nc.sync.dma_start(cos_sb, cosd.rearrange("p (b d) -> p b d", d=DH))
        nc.sync.dma_start(sin_sb, sind.rearrange("p (b d) -> p b d", d=DH))
        nc.sync.dma_start(mask_sb, maskd.rearrange("p (r n) -> p r n", n=512))
        ident = const.tile([128, 128], bf16)
        make_identity(nc, ident)
        ones_row = const.tile([33, 64], f32)
        nc.vector.memset(ones_row, 1.0)

        # persistent activations (qT doubles as attention-output storage)
        qT = [qkv.tile([128, S], bf16, tag=f"qT{e}", name=f"qT{e}")
              for e in range(ET)]
        kT = [qkv.tile([128, S], bf16, tag=f"kT{e}", name=f"kT{e}")
              for e in range(ET)]
        v_sb = qkv.tile([128, SS, HPC, 65], bf16, tag="v")

        # weights: 4 distinct slots, all loaded up front
        wq_sb = wpool.tile([128, DT, E], bf16, tag="w")
        wk_sb = wpool.tile([128, DT, E], bf16, tag="w")
        wv_sb = wpool.tile([128, DT, E], bf16, tag="w")
        wo_sb = wpool.tile([128, ET, DIM], bf16, tag="w")
        nc.sync.dma_start(wq_sb, wqT.rearrange("(t p) e -> p t e", p=128))
        nc.sync.dma_start(wk_sb, wkT.rearrange("(t p) e -> p t e", p=128))
        nc.sync.dma_start(wv_sb, wvT.rearrange("(t p) e -> p t e", p=128))
        nc.sync.dma_start(wo_sb, woT.rearrange("(t p) e -> p t e", p=128))
        # full x resident in SBUF: [p, dt, s] = xT[dt*128+p, s]
        xfull = xpool.tile([128, DT, S], bf16, tag="xf")
        for dt0 in range(DT):
            nc.sync.dma_start(xfull[:, dt0, :],
                              xT[dt0 * 128:(dt0 + 1) * 128, :])
        nc.vector.memset(v_sb[:, :, :, 64:65], 1.0)

        ps_pr_ref = [None]
        ps_lg_ref = [None]
        ps_pv_ref = [None]
        ps_y_ref = [None]

        def norm_rope_transpose(ps, dstT, st, su):
            """psum [si,e] natural -> per-head l2norm, rope, bf16, -> dstT."""
            sq = work.tile([128, E], f32, tag="sq", bufs=2)
            nc.scalar.square(sq, ps)
            ssq = work.tile([128, HPC], f32, tag="ssq", bufs=1)
            nc.vector.tensor_reduce(
                ssq, sq.rearrange("p (h d) -> p h d", d=DH),
                axis=mybir.AxisListType.X, op=mybir.AluOpType.add)
            nc.scalar.sqrt(ssq, ssq)
            inv = work.tile([128, HPC], f32, tag="inv", bufs=1)
            nc.vector.reciprocal(inv, ssq)
            qn = work.tile([128, HPC, DH], f32, tag="sq", bufs=2)
            nc.vector.tensor_mul(
                qn, ps.rearrange("p (h d) -> p h d", d=DH),
                inv.unsqueeze(2).broadcast_to([128, HPC, DH]))
            sblk = st * 4 + su
            cosb = cos_sb[:, sblk:sblk + 1, :].broadcast_to([128, HPC, DH])
            sinb = sin_sb[:, sblk:sblk + 1, :].broadcast_to([128, HPC, DH])
            rot = work.tile([128, HPC, 2, 32], f32, tag="rot", bufs=1)
            qn4 = qn.rearrange("p h (t u) -> p h t u", u=32)
            nc.vector.tensor_copy(rot[:, :, 0:1, :], qn4[:, :, 1:2, :])
            nc.vector.tensor_copy(rot[:, :, 1:2, :], qn4[:, :, 0:1, :])
            nc.vector.tensor_mul(rot.rearrange("p h t u -> p h (t u)"),
                                 rot.rearrange("p h t u -> p h (t u)"), sinb)
            qf = work.tile([128, HPC, DH], f32, tag="qf", bufs=1)
            nc.vector.tensor_mul(qf, qn, cosb)
            qo = work.tile([128, E], bf16, tag="qo", bufs=1)
            nc.vector.tensor_add(
                qo, qf.rearrange("p h d -> p (h d)"),
                rot.rearrange("p h t u -> p (h t u)"))
            for et in range(ET):
                trp = ps_pr_ref[0].tile([128, 128], bf16, tag="pr")
                nc.tensor.transpose(trp, qo[:, et * 128:(et + 1) * 128], ident)
                nc.vector.tensor_copy(
                    dstT[et][:, sblk * 128:(sblk + 1) * 128], trp)

        def proj_wave(w_sb, dstT, st, sus):
            """project si-128 subblocks `sus` of si-512 block st."""
            pss = [ps_pr_ref[0].tile([128, E], f32, tag="pr", name=f"ps{st}_{su}")
                   for su in sus]
            for dt in range(DT):
                for ci, su in enumerate(sus):
                    nc.tensor.matmul(
                        pss[ci],
                        xfull[:, dt, st * 512 + su * 128:
                              st * 512 + (su + 1) * 128],
                        w_sb[:, dt, :],
                        start=(dt == 0), stop=(dt == DT - 1))
            for ci, su in enumerate(sus):
                if dstT is None:
                    nc.vector.tensor_copy(
                        v_sb[:, st * 4 + su, :, 0:64],
                        pss[ci].rearrange("p (h d) -> p h d", d=DH))
                else:
                    norm_rope_transpose(pss[ci], dstT, st, su)

        def attn_head(h, i, pv):
            """head h, si-512 block i; returns held pv psum (unnormalized)."""
            et, hp = h // 2, (h % 2) * 64
            npr = 2 * (i + 1)    # lg pairs
            lgs = {}

            def emit_lg(p):
                lg2 = ps_lg_ref[0].tile([128, 2, 512], f32, tag="lg", name=f"lg{p}")
                for b in range(2):
                    sjb = 2 * p + b
                    nc.tensor.matmul(
                        lg2[:, b, :],
                        kT[et][hp:hp + 64, sjb * 128:(sjb + 1) * 128],
                        qT[et][hp:hp + 64, i * 512:(i + 1) * 512],
                        start=True, stop=True)
                lgs[p] = lg2

            emit_lg(0)
            if npr > 1:
                emit_lg(1)
            for p in range(npr):
                lg2 = lgs.pop(p)
                ex = attnp.tile([128, 2, 512], bf16, tag="ex", bufs=2)
                nc.scalar.activation(ex, lg2, AF.Exp)
                if p + 2 < npr:
                    emit_lg(p + 2)
                for b in range(2):
                    sjb = 2 * p + b
                    r = sjb - 4 * i
                    if r >= 0:
                        nc.vector.tensor_mul(ex[:, b, :], ex[:, b, :],
                                             mask_sb[:, r, :])
                    nc.tensor.matmul(
                        pv[0:65, :],
                        v_sb[:, sjb, h, :],
                        ex[:, b, :],
                        start=(sjb == 0), stop=(sjb == 4 * i + 3))

        def normalize_pair(h0, i, pvs, dn2):
            """divide outT of heads h0,h0+1 by softmax denominators."""
            inv2 = dn2
            nc.vector.reciprocal(inv2[0:33, :], dn2[0:33, :])
            for j, h in enumerate((h0, h0 + 1)):
                et, hp = h // 2, (h % 2) * 64
                bc = ps_lg_ref[0].tile([64, 512], f32, tag="lg")
                nc.tensor.matmul(bc, ones_row[32 * j:32 * j + 1, :],
                                 inv2[32 * j:32 * j + 1, :],
                                 start=True, stop=True)
                bcs = attnp.tile([64, 512], f32, tag="bcs", bufs=1)
                nc.scalar.copy(bcs, bc)
                nc.vector.tensor_mul(
                    qT[et][hp:hp + 64, i * 512:(i + 1) * 512],
                    pvs[j][0:64, :], bcs)

        def attn_block(i):
            pending = None

            def run_head(h):
                pv = ps_pv_ref[0].tile([128, 512], f32, tag="pv", name=f"pv{h}")
                attn_head(h, i, pv)
                return pv

            for h0 in range(0, HPC, 2):
                dn2 = attnp.tile([33, 512], f32, tag="dn2", bufs=2)
                nc.vector.memset(dn2, 1.0)
                pv0 = run_head(h0)
                nc.vector.tensor_copy(dn2[0:1, :], pv0[64:65, :])
                if pending is not None:
                    normalize_pair(*pending)
                    pending = None
                pv1 = run_head(h0 + 1)
                nc.vector.tensor_copy(dn2[32:33, :], pv1[64:65, :])
                pending = (h0, i, (pv0, pv1), dn2)
            normalize_pair(*pending)

        def yproj(ib):
            """si-128 block ib -> Y[ib*128:+128, :]."""
            for nd in range(4):
                ps = ps_y_ref[0].tile([128, 512], f32, tag="yps")
                for ket in range(ET):
                    nc.tensor.matmul(
                        ps,
                        qT[ket][:, ib * 128:(ib + 1) * 128],
                        wo_sb[:, ket, nd * 512:(nd + 1) * 512],
                        start=(ket == 0), stop=(ket == ET - 1))
                ys = ypool.tile([128, 512], f32, tag="y")
                nc.scalar.copy(ys, ps)
                nc.sync.dma_start(
                    Y[ib * 128:(ib + 1) * 128, nd * 512:(nd + 1) * 512], ys)

        with tc.tile_pool(name="ps_pr", bufs=8, space="PSUM") as pr_pool:
            ps_pr_ref[0] = pr_pool
            for st in range(SB):
                proj_wave(wq_sb, qT, st, (0, 1, 2, 3))
                proj_wave(wk_sb, kT, st, (0, 1, 2, 3))
                proj_wave(wv_sb, None, st, (0, 1, 2, 3))
        with tc.tile_pool(name="ps_lg", bufs=2, space="PSUM") as lg_pool, \
             tc.tile_pool(name="ps_pv", bufs=3, space="PSUM") as pv_pool, \
             tc.tile_pool(name="ps_y", bufs=1, space="PSUM") as y_pool:
            ps_lg_ref[0] = lg_pool
            ps_pv_ref[0] = pv_pool
            ps_y_ref[0] = y_pool
            for i in range(SB):
                attn_block(i)
                for ib in range(4 * i, 4 * i + 4):
                    yproj(ib)

    return nc


def _host_prep(x, wq, wk, wv, wo, qk_scale):
    """Returns per-core input dicts."""
    perm = np.concatenate([np.arange(0, DH, 2), np.arange(1, DH, 2)])
    wq_n = _l2n(wq, -1).reshape(HEADS, DH, DIM)[:, perm, :].reshape(HEADS * DH, DIM)
    wk_n = _l2n(wk, -1).reshape(HEADS, DH, DIM)[:, perm, :].reshape(HEADS * DH, DIM)
    wv_n = _l2n(wv, -1)
    wo_n = _l2n(wo, 0)
    sp = qk_scale.astype(np.float64)[perm]

    # rope tables with qk_scale folded in; permuted-block layout
    half = np.arange(0, DH, 2)
    freqs = 1.0 / (THETA ** (half.astype(np.float64) / DH))      # (32,)
    ang = np.arange(S, dtype=np.float64)[:, None] * freqs[None]  # (S, 32)
    cos_h, sin_h = np.cos(ang), np.sin(ang)
    cos_p = np.concatenate([cos_h, cos_h], 1)                    # (S, 64)
    sin_e = np.concatenate([-sin_h, sin_h], 1)
    cos_eff = (cos_p * sp[None, :]).astype(np.float32)
    swap_sp = np.concatenate([sp[32:], sp[:32]])
    sin_eff = (sin_e * swap_sp[None, :]).astype(np.float32)
    # device layout [128, SS*DH]: [p, b*64+c] = tbl[b*128+p, c]
    cosd = np.ascontiguousarray(
        cos_eff.reshape(SS, 128, DH).transpose(1, 0, 2).reshape(128, SS * DH))
    sind = np.ascontiguousarray(
        sin_eff.reshape(SS, 128, DH).transpose(1, 0, 2).reshape(128, SS * DH))

    # causal masks for the 4 diagonal offsets: keep sjl + 128r <= sil
    sjl = np.arange(128)[:, None]
    sil = np.arange(512)[None, :]
    maskd = np.ascontiguousarray(np.concatenate(
        [(sjl + 128 * r <= sil).astype(np.float32) for r in range(4)],
        axis=1))  # [128, 4*512]

    in_maps = []
    for c in range(NCORES):
        b, t = divmod(c, TP)
        e0 = t * E
        in_maps.append({
            "xT": np.ascontiguousarray(x[b].T).astype(BF16),
            "wqT": np.ascontiguousarray(wq_n[e0:e0 + E].T).astype(BF16),
            "wkT": np.ascontiguousarray(wk_n[e0:e0 + E].T).astype(BF16),
            "wvT": np.ascontiguousarray(wv_n[e0:e0 + E].T).astype(BF16),
            "woT": np.ascontiguousarray(wo_n[:, e0:e0 + E].T).astype(BF16),
            "cosd": cosd.astype(BF16), "sind": sind.astype(BF16),
            "maskd": maskd.astype(BF16),
        })
    return in_maps


def _install_profile_hook():
    """antenv.axon_hooks is absent in this image; shim it and register the
    ctypes NTFF hook against /opt/axon/libaxon_pjrt.so (mirrors trn_boot)."""
    import types
    import ctypes
    import contextlib

    try:
        from antenv.axon_hooks import get_axon_ntff_profile_hook  # noqa
        return
    except ImportError:
        pass
    import antenv
    mod = types.ModuleType("antenv.axon_hooks")
    state = {}
    mod.set_axon_ntff_profile_hook = lambda h: state.__setitem__("h", h)
    mod.get_axon_ntff_profile_hook = lambda: state.get("h")
    sys.modules["antenv.axon_hooks"] = mod
    antenv.axon_hooks = mod

    so_path = "/opt/axon/libaxon_pjrt.so"
    lib = ctypes.CDLL(so_path)
    if not hasattr(lib, "axon_start_nrt_profile"):
        return
    lib.axon_start_nrt_profile.argtypes = [
        ctypes.POINTER(ctypes.c_int64), ctypes.c_size_t]
    lib.axon_start_nrt_profile.restype = ctypes.c_int64
    lib.axon_stop_nrt_profile.argtypes = [ctypes.c_char_p]
    lib.axon_stop_nrt_profile.restype = ctypes.c_int64

    @contextlib.contextmanager
    def _hook(output_dir, device_ids):
        import jax
        jax.devices()
        if device_ids:
            ids = (ctypes.c_int64 * len(device_ids))(*device_ids)
            rc = lib.axon_start_nrt_profile(ids, len(device_ids))
        else:
            rc = lib.axon_start_nrt_profile(None, 0)
        if rc != 0:
            raise RuntimeError(f"axon_start_nrt_profile rc={rc}")
        try:
            yield
        finally:
            n = lib.axon_stop_nrt_profile(str(output_dir).encode())
            print(f"profile: {n} file(s) written to {output_dir}",
                  file=sys.stderr)

    mod.set_axon_ntff_profile_hook(_hook)


def kernel(x, wq, wk, wv, wo, qk_scale, _profile=False):
    from concourse.bass_utils import run_bass_kernel_spmd

    if _profile:
        _install_profile_hook()

    if "nc" not in _CACHE:
        nc = _build_program()
        nc.finalize()
        _CACHE["nc"] = nc
    nc = _CACHE["nc"]
    in_maps = _host_prep(np.asarray(x), np.asarray(wq), np.asarray(wk),
                         np.asarray(wv), np.asarray(wo), np.asarray(qk_scale))
    res = run_bass_kernel_spmd(nc, in_maps, core_ids=list(range(NCORES)),
                               trace=_profile)
    outs = res.results
    y = np.empty((B, S, DIM), dtype=np.float32)
    for b in range(B):
        y[b] = sum(outs[b * TP + t]["Y"] for t in range(TP))
    if _profile:
        _CACHE["last_exec_time_ns"] = res.exec_time_ns
        _CACHE["last_profile"] = res.profile_json
    return y



# revision 8
# speedup vs baseline: 1.2427x; 1.2427x over previous
"""nn_Attention Trainium2 Bass kernel (v2 — interleaved pipeline).

Full attention forward: x->(q,k,v) with l2-normalized weights, per-head-dim
l2 norm + learned qk scale, interleaved RoPE, causal SDPA, output projection
with column-l2-normalized wo.

Sharding: TP=4 over heads (8 heads/core) x DP=2 over batch across 8 cores.
Each core computes a partial [2048, 2048] output for its batch; host sums
the 4 TP partials per batch.

v2 changes vs v1:
- single interleaved loop per 512-row block: proj -> attention -> yproj,
  so DVE rope work, Act exp work and PE matmuls overlap across phases.
- q/k transposes via DMA xbar (dma_start_transpose) instead of PE
  transposes + DVE copies.
- causal mask as a single 128x128 triangle multiply on the Pool engine.
- lg/pv matmuls trimmed to the live columns on diagonal blocks.
- softmax denominators: v's 65th ones-column -> psum row 64 -> stashed ->
  gathered by DMA -> PE-transposed to si-partition layout -> one cheap
  [128,32] reciprocal -> transposed back -> rank-8 indicator matmul
  broadcast (replaces 3.3us-per-call wide DVE reciprocals).
- x streamed per 512-column block (2-deep) instead of fully resident.
- yproj results DMA'd directly from PSUM to DRAM.
"""
import sys
import os
import math
from contextlib import ExitStack

sys.path.insert(0, "/opt/trn_rl_repo")

import numpy as np
import ml_dtypes

BF16 = ml_dtypes.bfloat16

B, S, DIM = 2, 2048, 2048
HEADS, DH = 32, 64
THETA = 10000.0
NCORES = 8
TP = 4             # head-parallel ways
HPC = HEADS // TP  # heads per core = 8
E = HPC * DH       # per-core qkv width = 512
ET = E // 128      # e-tiles per core = 4
DT = DIM // 128    # contraction d-tiles = 16
SB = S // 512      # 512-wide seq blocks = 4
SS = S // 128      # 128-wide seq blocks = 16

_CACHE = {}


def _l2n(w, axis):
    n = np.sqrt((w.astype(np.float64) ** 2).sum(axis=axis, keepdims=True))
    n = np.maximum(n, 1e-12)
    return (w / n).astype(np.float32)


def _build_program():
    import concourse.bass as bass
    from concourse import bacc
    import concourse.mybir as mybir
    import concourse.tile as tile
    from concourse.masks import make_identity

    f32 = mybir.dt.float32
    bf16 = mybir.dt.bfloat16
    AF = mybir.ActivationFunctionType
    AX = mybir.AxisListType
    OP = mybir.AluOpType

    nc = bacc.Bacc("TRN2", target_bir_lowering=False)

    xT = nc.dram_tensor("xT", [DIM, S], bf16, kind="ExternalInput")
    wqT = nc.dram_tensor("wqT", [DIM, E], bf16, kind="ExternalInput")
    wkT = nc.dram_tensor("wkT", [DIM, E], bf16, kind="ExternalInput")
    wvT = nc.dram_tensor("wvT", [DIM, E], bf16, kind="ExternalInput")
    woT = nc.dram_tensor("woT", [E, DIM], bf16, kind="ExternalInput")
    cosd = nc.dram_tensor("cosd", [128, SS * DH], bf16, kind="ExternalInput")
    sind = nc.dram_tensor("sind", [128, SS * DH], bf16, kind="ExternalInput")
    trid = nc.dram_tensor("trid", [128, 128], bf16, kind="ExternalInput")
    ind8d = nc.dram_tensor("ind8d", [8, 512], bf16, kind="ExternalInput")
    Y = nc.dram_tensor("Y", [S, DIM], f32, kind="ExternalOutput")

    with tile.TileContext(nc) as tc, ExitStack() as ctx:
        const = ctx.enter_context(tc.tile_pool(name="const", bufs=1))
        wpool = ctx.enter_context(tc.tile_pool(name="wpool", bufs=4))
        xpool = ctx.enter_context(tc.tile_pool(name="xpool", bufs=2))
        qkv = ctx.enter_context(tc.tile_pool(name="qkv", bufs=1))
        work = ctx.enter_context(tc.tile_pool(name="work", bufs=1))
        expool = ctx.enter_context(tc.tile_pool(name="expool", bufs=4))
        psA = ctx.enter_context(
            tc.tile_pool(name="psA", bufs=4, space="PSUM"))
        psL = ctx.enter_context(
            tc.tile_pool(name="psL", bufs=2, space="PSUM"))

        # --- weights (wq first, quartered, so proj can start early) ---
        wq_sb = wpool.tile([128, DT, E], bf16, tag="w")
        wk_sb = wpool.tile([128, DT, E], bf16, tag="w")
        wv_sb = wpool.tile([128, DT, E], bf16, tag="w")
        wo_sb = wpool.tile([128, ET, DIM], bf16, tag="w")
        wqr = wqT.rearrange("(t p) e -> p t e", p=128)
        for q4 in range(4):
            nc.sync.dma_start(wq_sb[:, q4 * 4:(q4 + 1) * 4, :],
                              wqr[:, q4 * 4:(q4 + 1) * 4, :])

        xtiles = {}

        def load_x(st):
            t = xpool.tile([128, DT, 512], bf16, tag="xst", name=f"xst{st}")
            src = xT[:, st * 512:(st + 1) * 512].rearrange(
                "(t p) s -> p t s", p=128)
            for q4 in range(4):
                nc.sync.dma_start(t[:, q4 * 4:(q4 + 1) * 4, :],
                                  src[:, q4 * 4:(q4 + 1) * 4, :])
            return t

        xtiles[0] = load_x(0)
        nc.sync.dma_start(wk_sb, wkT.rearrange("(t p) e -> p t e", p=128))
        nc.sync.dma_start(wv_sb, wvT.rearrange("(t p) e -> p t e", p=128))

        # --- constants ---
        cos_sb = const.tile([128, SS, DH], bf16)
        sin_sb = const.tile([128, SS, DH], bf16)
        nc.sync.dma_start(cos_sb, cosd.rearrange("p (b d) -> p b d", d=DH))
        nc.sync.dma_start(sin_sb, sind.rearrange("p (b d) -> p b d", d=DH))
        tri = const.tile([128, 128], bf16)
        nc.sync.dma_start(tri, trid[:, :])
        ind8 = const.tile([8, 512], bf16)
        nc.sync.dma_start(ind8, ind8d[:, :])
        nc.sync.dma_start(wo_sb, woT.rearrange("(t p) e -> p t e", p=128))
        identf = const.tile([128, 128], f32)
        make_identity(nc, identf)

        # --- persistent activations ---
        qTall = qkv.tile([128, ET, S], bf16, tag="qT")
        kTall = qkv.tile([128, ET, S], bf16, tag="kT")
        v_sb = qkv.tile([128, SS, HPC, 65], bf16, tag="v")
        stash = qkv.tile([65, HPC, 512], f32, tag="stash")
        nc.vector.memset(v_sb[:, :, :, 64:65], 1.0)

        def norm_rope(ps, dstT, st, su):
            """psum [si,e] natural -> per-head l2norm, rope, bf16,
            -> DMA-transpose into dstT columns."""
            sblk = st * 4 + su
            sq = work.tile([128, E], f32, tag="sq", bufs=2)
            nc.scalar.square(sq, ps)
            ssq = work.tile([128, HPC], f32, tag="ssq", bufs=2)
            nc.vector.tensor_reduce(
                ssq, sq.rearrange("p (h d) -> p h d", d=DH),
                axis=AX.X, op=OP.add)
            nc.scalar.sqrt(ssq, ssq)
            inv = work.tile([128, HPC], f32, tag="inv", bufs=2)
            nc.vector.reciprocal(inv, ssq)
            qn = work.tile([128, HPC, DH], f32, tag="qn", bufs=2)
            nc.vector.tensor_mul(
                qn, ps.rearrange("p (h d) -> p h d", d=DH),
                inv.unsqueeze(2).broadcast_to([128, HPC, DH]))
            cosb = cos_sb[:, sblk:sblk + 1, :].broadcast_to([128, HPC, DH])
            sinb = sin_sb[:, sblk:sblk + 1, :].broadcast_to([128, HPC, DH])
            rot = work.tile([128, HPC, 2, 32], f32, tag="rot", bufs=2)
            qn4 = qn.rearrange("p h (t u) -> p h t u", u=32)
            nc.gpsimd.tensor_copy(rot[:, :, 0:1, :], qn4[:, :, 1:2, :])
            nc.gpsimd.tensor_copy(rot[:, :, 1:2, :], qn4[:, :, 0:1, :])
            nc.gpsimd.tensor_mul(rot.rearrange("p h t u -> p h (t u)"),
                                 rot.rearrange("p h t u -> p h (t u)"), sinb)
            qf = work.tile([128, HPC, DH], f32, tag="qf", bufs=2)
            nc.gpsimd.tensor_mul(qf, qn, cosb)
            qo = work.tile([128, E], bf16, tag="qo", bufs=3)
            nc.vector.tensor_add(
                qo, qf.rearrange("p h d -> p (h d)"),
                rot.rearrange("p h t u -> p (h t u)"))
            nc.sync.dma_start_transpose(
                dstT[:, :, sblk * 128:(sblk + 1) * 128], qo)

        def proj_wave(w_sb, kind, st, xt):
            pss = [psA.tile([128, E], f32, tag="ps", name=f"p{kind}{st}_{su}")
                   for su in range(4)]
            for dt in range(DT):
                for su in range(4):
                    nc.tensor.matmul(
                        pss[su],
                        xt[:, dt, su * 128:(su + 1) * 128],
                        w_sb[:, dt, :],
                        start=(dt == 0), stop=(dt == DT - 1))
            for su in range(4):
                if kind == "v":
                    nc.vector.tensor_copy(
                        v_sb[:, st * 4 + su, :, 0:64],
                        pss[su].rearrange("p (h d) -> p h d", d=DH))
                else:
                    norm_rope(pss[su], qTall if kind == "q" else kTall,
                              st, su)

        def proj_all(st):
            xt = xtiles[st]
            proj_wave(wq_sb, "q", st, xt)
            proj_wave(wk_sb, "k", st, xt)
            proj_wave(wv_sb, "v", st, xt)

        def attn_block(i):
            last = 4 * i + 3
            for h in range(HPC):
                et, hp = h // 2, (h % 2) * 64
                pv = psA.tile([128, 512], f32, tag="ps", name=f"pv{i}_{h}")
                npr = 2 * (i + 1)
                lgs = {}

                def emit_lg(p):
                    lg2 = psL.tile([128, 2, 512], f32, tag="lg",
                                   name=f"lg{i}_{h}_{p}")
                    for b in range(2):
                        sjb = 2 * p + b
                        r = sjb - 4 * i
                        c0 = r * 128 if r > 0 else 0
                        nc.tensor.matmul(
                            lg2[:, b, c0:],
                            kTall[hp:hp + 64, et, sjb * 128:(sjb + 1) * 128],
                            qTall[hp:hp + 64, et,
                                  i * 512 + c0:(i + 1) * 512],
                            start=True, stop=True)
                    lgs[p] = lg2

                emit_lg(0)
                if npr > 1:
                    emit_lg(1)
                for p in range(npr):
                    lg2 = lgs.pop(p)
                    ex = expool.tile([128, 2, 512], bf16, tag="ex")
                    if 2 * p - 4 * i >= 0:  # diagonal pair: match lg trim
                        for b in range(2):
                            c0 = max(0, (2 * p + b - 4 * i)) * 128
                            nc.scalar.activation(ex[:, b, c0:],
                                                 lg2[:, b, c0:], AF.Exp)
                    else:
                        nc.scalar.activation(ex, lg2, AF.Exp)
                    if p + 2 < npr:
                        emit_lg(p + 2)
                    for b in range(2):
                        sjb = 2 * p + b
                        r = sjb - 4 * i
                        if r >= 0:
                            nc.gpsimd.tensor_mul(
                                ex[:, b, r * 128:(r + 1) * 128],
                                ex[:, b, r * 128:(r + 1) * 128], tri)
                        c0 = r * 128 if r > 0 else 0
                        nc.tensor.matmul(
                            pv[0:65, c0:],
                            v_sb[:, sjb, h, :],
                            ex[:, b, c0:],
                            start=(sjb == 0), stop=(sjb == last))
                nc.vector.tensor_copy(stash[:, h, :], pv[0:65, :])

        def normalize(i):
            """1/den in si-partition layout via PE transposes, then rank-8
            indicator broadcast + per-head mul into qTall."""
            den = work.tile([8, 512], f32, tag="den", bufs=2)
            nc.sync.dma_start(den, stash[64:65, :, :])
            invT = psA.tile([128, 32], f32, tag="ps")
            for c in range(4):
                nc.tensor.transpose(
                    invT[:, c * 8:(c + 1) * 8],
                    den[:, c * 128:(c + 1) * 128], identf[0:8, 0:8])
            inv_sb = work.tile([128, 32], f32, tag="invsb", bufs=2)
            nc.vector.reciprocal(inv_sb, invT)
            invrow = psA.tile([8, 4, 128], f32, tag="ps")
            for c in range(4):
                nc.tensor.transpose(
                    invrow[:, c, :], inv_sb[:, c * 8:(c + 1) * 8], identf)
            inv_row = work.tile([8, 512], bf16, tag="invrowsb", bufs=2)
            nc.vector.tensor_copy(
                inv_row, invrow.rearrange("p c j -> p (c j)"))
            for h in range(HPC):
                et, hp = h // 2, (h % 2) * 64
                bc = psA.tile([64, 512], f32, tag="ps", name=f"bc{i}_{h}")
                nc.tensor.matmul(bc, ind8[:, h * 64:(h + 1) * 64], inv_row,
                                 start=True, stop=True)
                nc.vector.tensor_mul(
                    qTall[hp:hp + 64, et, i * 512:(i + 1) * 512],
                    stash[0:64, h, :], bc)

        def yproj_block(i):
            for ib in range(4 * i, 4 * i + 4):
                for nd in range(4):
                    ps = psA.tile([128, 512], f32, tag="ps",
                                  name=f"y{ib}_{nd}")
                    for ket in range(ET):
                        nc.tensor.matmul(
                            ps,
                            qTall[:, ket, ib * 128:(ib + 1) * 128],
                            wo_sb[:, ket, nd * 512:(nd + 1) * 512],
                            start=(ket == 0), stop=(ket == ET - 1))
                    ys = work.tile([128, 512], f32, tag="ys", bufs=3)
                    if nd % 2 == 0:
                        nc.vector.tensor_copy(ys, ps)
                    else:
                        nc.scalar.copy(ys, ps)
                    nc.sync.dma_start(
                        Y[ib * 128:(ib + 1) * 128, nd * 512:(nd + 1) * 512],
                        ys)

        proj_all(0)
        for st in range(SB):
            if st + 1 < SB:
                xtiles[st + 1] = load_x(st + 1)
            attn_block(st)
            normalize(st)
            yproj_block(st)
            if st + 1 < SB:
                proj_all(st + 1)

    return nc


def _host_prep(x, wq, wk, wv, wo, qk_scale):
    """Returns per-core input dicts."""
    perm = np.concatenate([np.arange(0, DH, 2), np.arange(1, DH, 2)])
    wq_n = _l2n(wq, -1).reshape(HEADS, DH, DIM)[:, perm, :].reshape(HEADS * DH, DIM)
    wk_n = _l2n(wk, -1).reshape(HEADS, DH, DIM)[:, perm, :].reshape(HEADS * DH, DIM)
    wv_n = _l2n(wv, -1)
    wo_n = _l2n(wo, 0)
    sp = qk_scale.astype(np.float64)[perm]

    # rope tables with qk_scale folded in; permuted-block layout
    half = np.arange(0, DH, 2)
    freqs = 1.0 / (THETA ** (half.astype(np.float64) / DH))      # (32,)
    ang = np.arange(S, dtype=np.float64)[:, None] * freqs[None]  # (S, 32)
    cos_h, sin_h = np.cos(ang), np.sin(ang)
    cos_p = np.concatenate([cos_h, cos_h], 1)                    # (S, 64)
    sin_e = np.concatenate([-sin_h, sin_h], 1)
    cos_eff = (cos_p * sp[None, :]).astype(np.float32)
    swap_sp = np.concatenate([sp[32:], sp[:32]])
    sin_eff = (sin_e * swap_sp[None, :]).astype(np.float32)
    # device layout [128, SS*DH]: [p, b*64+c] = tbl[b*128+p, c]
    cosd = np.ascontiguousarray(
        cos_eff.reshape(SS, 128, DH).transpose(1, 0, 2).reshape(128, SS * DH))
    sind = np.ascontiguousarray(
        sin_eff.reshape(SS, 128, DH).transpose(1, 0, 2).reshape(128, SS * DH))

    # causal triangle for the diagonal 128-blocks: keep sjl <= sil
    sjl = np.arange(128)[:, None]
    sil = np.arange(128)[None, :]
    trid = (sjl <= sil).astype(np.float32)

    # indicator for denominator broadcast: ind8[k, h*64+m] = (k == h)
    ind8 = np.zeros((8, 512), dtype=np.float32)
    for h in range(8):
        ind8[h, h * 64:(h + 1) * 64] = 1.0

    in_maps = []
    for c in range(NCORES):
        b, t = divmod(c, TP)
        e0 = t * E
        in_maps.append({
            "xT": np.ascontiguousarray(x[b].T).astype(BF16),
            "wqT": np.ascontiguousarray(wq_n[e0:e0 + E].T).astype(BF16),
            "wkT": np.ascontiguousarray(wk_n[e0:e0 + E].T).astype(BF16),
            "wvT": np.ascontiguousarray(wv_n[e0:e0 + E].T).astype(BF16),
            "woT": np.ascontiguousarray(wo_n[:, e0:e0 + E].T).astype(BF16),
            "cosd": cosd.astype(BF16), "sind": sind.astype(BF16),
            "trid": trid.astype(BF16), "ind8d": ind8.astype(BF16),
        })
    return in_maps


def _install_profile_hook():
    """antenv.axon_hooks is absent in this image; shim it and register the
    ctypes NTFF hook against /opt/axon/libaxon_pjrt.so (mirrors trn_boot)."""
    import types
    import ctypes
    import contextlib

    try:
        from antenv.axon_hooks import get_axon_ntff_profile_hook  # noqa
        return
    except ImportError:
        pass
    import antenv
    mod = types.ModuleType("antenv.axon_hooks")
    state = {}
    mod.set_axon_ntff_profile_hook = lambda h: state.__setitem__("h", h)
    mod.get_axon_ntff_profile_hook = lambda: state.get("h")
    sys.modules["antenv.axon_hooks"] = mod
    antenv.axon_hooks = mod

    so_path = "/opt/axon/libaxon_pjrt.so"
    lib = ctypes.CDLL(so_path)
    if not hasattr(lib, "axon_start_nrt_profile"):
        return
    lib.axon_start_nrt_profile.argtypes = [
        ctypes.POINTER(ctypes.c_int64), ctypes.c_size_t]
    lib.axon_start_nrt_profile.restype = ctypes.c_int64
    lib.axon_stop_nrt_profile.argtypes = [ctypes.c_char_p]
    lib.axon_stop_nrt_profile.restype = ctypes.c_int64

    @contextlib.contextmanager
    def _hook(output_dir, device_ids):
        import jax
        jax.devices()
        if device_ids:
            ids = (ctypes.c_int64 * len(device_ids))(*device_ids)
            rc = lib.axon_start_nrt_profile(ids, len(device_ids))
        else:
            rc = lib.axon_start_nrt_profile(None, 0)
        if rc != 0:
            raise RuntimeError(f"axon_start_nrt_profile rc={rc}")
        try:
            yield
        finally:
            n = lib.axon_stop_nrt_profile(str(output_dir).encode())
            print(f"profile: {n} file(s) written to {output_dir}",
                  file=sys.stderr)

    mod.set_axon_ntff_profile_hook(_hook)


def kernel(x, wq, wk, wv, wo, qk_scale, _profile=False):
    from concourse.bass_utils import run_bass_kernel_spmd

    if _profile:
        _install_profile_hook()

    if "nc" not in _CACHE:
        nc = _build_program()
        nc.finalize()
        _CACHE["nc"] = nc
    nc = _CACHE["nc"]
    in_maps = _host_prep(np.asarray(x), np.asarray(wq), np.asarray(wk),
                         np.asarray(wv), np.asarray(wo), np.asarray(qk_scale))
    res = run_bass_kernel_spmd(nc, in_maps, core_ids=list(range(NCORES)),
                               trace=_profile)
    outs = res.results
    y = np.empty((B, S, DIM), dtype=np.float32)
    for b in range(B):
        y[b] = sum(outs[b * TP + t]["Y"] for t in range(TP))
    if _profile:
        _CACHE["last_exec_time_ns"] = res.exec_time_ns
        _CACHE["last_profile"] = res.profile_json
    return y
